# revision 7
# baseline (speedup 1.0000x reference)
"""Binarized 3x3 conv (GeneralConv2d) on 8 NeuronCores, fp8 DoubleRow edition.

y[b,o,h,w] = mean_abs(w[o]) * sum_{c,kh,kw} sign(w[o,c,kh,kw]) * x[b,c,h+kh-1,w+kw-1]

Data-parallel over batch: 4 images per core on 8 cores.  Per core the conv
runs on the PE array as fp8e4 DoubleRow matmuls (two 128-deep contraction
groups per instruction at 0.5 cycles/row): x is split exactly into
x = hi + lo with hi = fp8(x), lo = fp8(x - hi), and each (in-chunk, tap)
matmul contracts the (hi, lo) pair in one DoubleRow op against sign
weights +-0.5 (stride-0 broadcast across the pair), so the PSUM result is
0.5*sign(w)^T (hi + lo); the eviction multiplies by 2*mean_abs(w)/CKK.
Borders use clipped matmuls onto a start-zeroed PSUM bank instead of
zero halos.  Weights arrive bf16 in a host-pretransposed [oo, ckk, o]
layout (data movement + dtype narrowing only; sign(bf16(w)) == sign(w)
for all |w| >= 1e-40, and mean|w| shifts by <0.1%); sign, scale, and
quantization all run on-device.

Schedule: a short dummy-matmul burst ramps the PE p-state before real
work arrives.  All DMAs issue from SP in a hand-interleaved order on the
single DMA engine.  Image 0 streams in 7 slabs (5-row first slab, two
4-row lead chunks) with conv emission interleaved into the load stream
so Act's zero-run-ahead queue sees ops in ready order; its evictions are
raw bf16 copies (no scale dependency, banks free immediately) fixed up
in place once the scale weights+reduction land after image 1's x.
Image-0/1 lo-subs on DVE, images 2-3 on Pool; hi-casts on Act;
images 1-3 evict scaled, alternating Act/DVE.
"""

import numpy as np

from contextlib import ExitStack

import concourse.bass as bass
import concourse.mybir as mybir
from concourse import bacc
import concourse.tile as tile

dt = mybir.dt
OUT_C, IN_C = 256, 256
KH = KW = 3
KK = KH * KW            # 9
P = 128
CC = IN_C // P          # 2 in-channel chunks
OO = OUT_C // P         # 2 out-channel chunks
CKK = IN_C * KK         # 2304
QKO = KK * P            # 1152 transposed cols per (oo, cc) quarter
DR = mybir.MatmulPerfMode.DoubleRow

BATCH, H, W = 32, 56, 56
HW = H * W
N_CORES = 8
IMGS = BATCH // N_CORES  # 4
HCH = 8                  # output rows per PSUM chunk (steady state)
NCH = H // HCH           # 7

# image-0 x slabs (row ranges) and chunk row ranges
SLABS0 = [(0, 5), (5, 13), (13, 21), (21, 29), (29, 37), (37, 45), (45, 56)]
CHUNKS0 = [(0, 4), (4, 8)] + [(8 * k, 8 * k + 8) for k in range(1, NCH)]


def _build(imgs=IMGS, psum_bufs=8, ostage=4):
    nc = bacc.Bacc("TRN2", target_bir_lowering=False, debug=False,
                   enable_asserts=False, num_devices=8)
    x = nc.declare_dram_parameter("x", [imgs, IN_C, HW], dt.float32, isOutput=False)
    w = nc.declare_dram_parameter("w", [OUT_C, CKK], dt.bfloat16, isOutput=False)
    wt = nc.declare_dram_parameter("wt", [OO, CKK, P], dt.bfloat16, isOutput=False)
    y = nc.declare_dram_parameter("y", [imgs, OUT_C, HW], dt.bfloat16, isOutput=True)

    # Transposed-weight view: [oo][c (partition, stride 9 rows)][cc][(k o) contig]
    wtv = wt.rearrange("oo (cc c k) o -> oo c cc (k o)", cc=CC, c=P, k=KK)

    with tile.TileContext(nc) as tc, ExitStack() as ctx:
        wp = ctx.enter_context(tc.tile_pool(name="wp", bufs=1))
        w_sb = wp.tile([P, OO, CKK], dt.bfloat16)         # original layout (scale)
        wt32 = wp.tile([P, OO, CC, QKO], dt.bfloat16)     # transposed
        wt8 = wp.tile([P, OO, CC, QKO], dt.float8e4)      # [c, oo, cc, (k o)]
        scale = wp.tile([P, OO], dt.float32)
        warm = wp.tile([P, 512], dt.bfloat16)
        wt8v = wt8.rearrange("p oo cc (k o) -> p oo cc k o", o=P)

        x32p = ctx.enter_context(tc.tile_pool(name="x32", bufs=2))
        xqp = ctx.enter_context(tc.tile_pool(name="xq", bufs=imgs))
        pp = ctx.enter_context(tc.tile_pool(name="ps", bufs=psum_bufs, space="PSUM"))
        op = ctx.enter_context(tc.tile_pool(name="st", bufs=ostage))
        rawp = ctx.enter_context(tc.tile_pool(name="raw", bufs=2 * len(CHUNKS0)))

        x32s, xqs = {}, {}

        def wdma(oo, cc):
            nc.sync.dma_start(out=wt32[:, oo, cc, :], in_=wtv[oo, :, cc, :])

        def wsgn(oo, cc):
            # sign as (w >= 0) - 0.5 in {-0.5, +0.5} (exact fp8)
            nc.vector.tensor_scalar(
                out=wt8[:, oo, cc, :], in0=wt32[:, oo, cc, :],
                scalar1=0.0, scalar2=0.5,
                op0=mybir.AluOpType.is_ge, op1=mybir.AluOpType.subtract)

        def wodma(oo, half):
            q = CKK // 2
            nc.sync.dma_start(out=w_sb[:, oo, half * q:(half + 1) * q],
                              in_=w[oo * P:(oo + 1) * P, half * q:(half + 1) * q])

        def reduce_scale(oo):
            nc.vector.tensor_reduce(
                out=scale[:, oo:oo + 1], in_=w_sb[:, oo, :],
                axis=mybir.AxisListType.X,
                op=mybir.AluOpType.add, apply_absolute_value=True)
            # x2 compensates the +-0.5 sign weights
            nc.vector.tensor_scalar_mul(scale[:, oo:oo + 1], scale[:, oo:oo + 1],
                                        2.0 / CKK)

        def xalloc(img):
            x32s[img] = x32p.tile([P, CC, HW], dt.float32, name=f"x32_{img}", tag="x32")
            xqs[img] = xqp.tile([P, 2, CC, H, W], dt.float8e4, name=f"xq_{img}", tag="xq")

        def xdma(img, r0, r1):
            nc.sync.dma_start(
                out=x32s[img][:, :, r0 * W:r1 * W],
                in_=x[img, :, r0 * W:r1 * W].rearrange("(cc p) hw -> p cc hw", cc=CC))

        def quant(img, r0, r1, sub_eng):
            xq, x32 = xqs[img], x32s[img]
            x4 = x32.rearrange("p cc (h w) -> p cc h w", w=W)
            nc.scalar.copy(out=xq[:, 0, :, r0:r1, :], in_=x4[:, :, r0:r1, :])
            sub_eng.tensor_sub(xq[:, 1, :, r0:r1, :], x4[:, :, r0:r1, :],
                               xq[:, 0, :, r0:r1, :])

        def matmuls(img, oo, r0, r1):
            xq = xqs[img]
            rr = r1 - r0
            ps = pp.tile([P, rr, W], dt.float32, name=f"ps_{img}_{oo}_{r0}", tag="ps")
            n, last = 0, CC * KK - 1
            for ki in range(KH):
                for cc in range(CC):
                    for kj in range(KW):
                        xr = r0 + ki - 1
                        ro = max(0, -xr)
                        rows = min(H, xr + rr) - (xr + ro)
                        xc = kj - 1
                        co = max(0, -xc)
                        cols = min(W, xc + W) - (xc + co)
                        nc.tensor.matmul(
                            ps[:, ro:ro + rows, co:co + cols],
                            lhsT=wt8v[:, oo, cc, ki * KW + kj, :]
                                .unsqueeze(1).broadcast_to([P, 2, P]),
                            rhs=xq[:, :, cc, xr + ro:xr + ro + rows,
                                   xc + co:xc + co + cols],
                            start=(n == 0), stop=(n == last),
                            perf_mode=DR, skip_group_check=True)
                        n += 1
            return ps.rearrange("p h w -> p (h w)")

        raw_sts = []

        def conv_chunk_raw(img, oo, r0, r1, eng):
            # evict unscaled (no scale dependency); fixed up in place later
            psf = matmuls(img, oo, r0, r1)
            st = rawp.tile([P, (r1 - r0) * W], dt.bfloat16,
                           name=f"raw_{img}_{oo}_{r0}", tag="raw")
            if eng is nc.scalar:
                eng.copy(out=st, in_=psf)
            else:
                eng.tensor_copy(out=st, in_=psf)
            raw_sts.append((img, oo, r0, r1, st))

        def fixup(img, oo, r0, r1, st, eng):
            if eng is nc.scalar:
                eng.mul(st, st, scale[:, oo:oo + 1])
            else:
                eng.tensor_scalar_mul(st, st, scale[:, oo:oo + 1])
            nc.sync.dma_start(
                out=y[img, oo * P:(oo + 1) * P, r0 * W:r1 * W], in_=st)

        evn = [0]

        def conv_chunk(img, oo, ih):
            psf = matmuls(img, oo, ih * HCH, ih * HCH + HCH)
            st = op.tile([P, HCH * W], dt.bfloat16, name=f"st_{img}_{oo}_{ih}", tag="st")
            eng = (nc.vector, nc.scalar)[evn[0] % 2]
            evn[0] += 1
            if eng is nc.scalar:
                eng.mul(st, psf, scale[:, oo:oo + 1])
            else:
                eng.tensor_scalar_mul(st, psf, scale[:, oo:oo + 1])
            nc.sync.dma_start(
                out=y[img, oo * P:(oo + 1) * P, ih * HCH * W:(ih + 1) * HCH * W],
                in_=st)

        # --- emission schedule ---
        # PE p-state warmup: dummy accumulation on a zeroed tile, result unread.
        nc.vector.memset(warm, 0.0)
        dps = pp.tile([P, 512], dt.float32, name="ps_warm", tag="ps")
        for i in range(4):
            nc.tensor.matmul(dps, lhsT=warm[:, 0:P], rhs=warm,
                             start=(i == 0), stop=(i == 3), skip_group_check=True)

        # image-0 load stream with conv pairs interleaved (Act/DVE queues stay
        # in ready order); weight tiles ride between the x slabs.
        xalloc(0)
        xdma(0, *SLABS0[0])
        wdma(0, 0)
        wdma(0, 1)
        wsgn(0, 0)
        quant(0, *SLABS0[0], nc.vector)
        wsgn(0, 1)
        xdma(0, *SLABS0[1])
        wdma(1, 0)
        quant(0, *SLABS0[1], nc.vector)
        wsgn(1, 0)
        for pair, (r0, r1) in enumerate(CHUNKS0):
            s = pair + 2
            if s < len(SLABS0):
                xdma(0, *SLABS0[s])
                if s == 2:
                    wdma(1, 1)
                quant(0, *SLABS0[s], nc.vector)
                if s == 2:
                    wsgn(1, 1)
            conv_chunk_raw(0, 0, r0, r1, nc.scalar)
            conv_chunk_raw(0, 1, r0, r1, nc.vector)

        # image 1 x + scale weights + reduction + image-0 fixups
        xalloc(1)
        for hf in range(2):
            xdma(1, 28 * hf, 28 * hf + 28)
            quant(1, 28 * hf, 28 * hf + 28, nc.vector)
        for oo in range(OO):
            for half in range(2):
                wodma(oo, half)
        reduce_scale(0)
        reduce_scale(1)
        for i, (img, oo, r0, r1, st) in enumerate(raw_sts):
            fixup(img, oo, r0, r1, st, (nc.scalar, nc.vector)[i % 2])

        for ih in range(NCH):
            conv_chunk(1, 0, ih)

        def load_img(img):
            xalloc(img)
            for hf in range(2):
                xdma(img, 28 * hf, 28 * hf + 28)
                quant(img, 28 * hf, 28 * hf + 28, nc.gpsimd)

        load_img(2)
        for ih in range(NCH):
            conv_chunk(1, 1, ih)
        load_img(3)
        for img in range(2, imgs):
            for oo in range(OO):
                for ih in range(NCH):
                    conv_chunk(img, oo, ih)
    nc.compile()
    return nc


_NC_CACHE = {}


def _get_nc():
    if "nc" not in _NC_CACHE:
        _NC_CACHE["nc"] = _build()
    return _NC_CACHE["nc"]


def kernel(**inputs) -> np.ndarray:
    import ml_dtypes
    from concourse.bass_utils import run_bass_kernel_spmd

    x = np.ascontiguousarray(np.asarray(inputs["x"], dtype=np.float32))
    weight = np.ascontiguousarray(np.asarray(inputs["weight"], dtype=np.float32))
    assert x.shape == (BATCH, IN_C, H, W), x.shape
    assert weight.shape == (OUT_C * CKK, 1), weight.shape

    bf16 = ml_dtypes.bfloat16
    w2d = np.ascontiguousarray(weight.reshape(OUT_C, CKK).astype(bf16))
    # host-side layout transpose + bf16 narrowing only: [OO, CKK, P]
    wtr = np.ascontiguousarray(
        weight.reshape(OO, P, CKK).transpose(0, 2, 1).astype(bf16))

    nc = _get_nc()
    xr = x.reshape(BATCH, IN_C, HW)
    in_maps = [
        {"x": xr[c * IMGS:(c + 1) * IMGS], "w": w2d, "wt": wtr}
        for c in range(N_CORES)
    ]
    res = run_bass_kernel_spmd(nc, in_maps, core_ids=list(range(N_CORES)))
    out = np.concatenate(
        [np.asarray(res.results[c]["y"]).astype(np.float32) for c in range(N_CORES)],
        axis=0)
    return out.reshape(BATCH, OUT_C, H, W)


# revision 8
# speedup vs baseline: 1.0207x; 1.0207x over previous
"""Binarized 3x3 conv (GeneralConv2d) on 8 NeuronCores, fp8 DoubleRow edition.

y[b,o,h,w] = mean_abs(w[o]) * sum_{c,kh,kw} sign(w[o,c,kh,kw]) * x[b,c,h+kh-1,w+kw-1]

Data-parallel over batch: 4 images per core on 8 cores.  Per core the conv
runs on the PE array as fp8e4 DoubleRow matmuls (two 128-deep contraction
groups per instruction at 0.5 cycles/row): x is split exactly into
x = hi + lo with hi = fp8(x), lo = fp8(x - hi), and each (in-chunk, tap)
matmul contracts the (hi, lo) pair in one DoubleRow op against sign
weights +-0.5 (stride-0 broadcast across the pair), so the PSUM result is
0.5*sign(w)^T (hi + lo); the eviction multiplies by 2*mean_abs(w)/CKK.
Borders use clipped matmuls onto a start-zeroed PSUM bank instead of
zero halos.  Weights arrive bf16 in a host-pretransposed [oo, ckk, o]
layout (data movement + dtype narrowing only; sign(bf16(w)) == sign(w)
for all |w| >= 1e-40, and mean|w| shifts by <0.1%); sign, scale, and
quantization all run on-device.

Schedule: a short dummy-matmul burst ramps the PE p-state before real
work arrives.  All DMAs issue from SP in a hand-interleaved order on the
single DMA engine.  Image 0 streams in 7 slabs (5-row first slab, two
4-row lead chunks) with conv emission interleaved into the load stream
so Act's zero-run-ahead queue sees ops in ready order; its evictions are
raw bf16 copies (no scale dependency, banks free immediately) fixed up
in place once the scale weights+reduction land after image 1's x.
Image-0/1 lo-subs on DVE, images 2-3 on Pool; hi-casts on Act;
images 1-3 evict scaled, alternating Act/DVE.
"""

import numpy as np

from contextlib import ExitStack

import concourse.bass as bass
import concourse.mybir as mybir
from concourse import bacc
import concourse.tile as tile

dt = mybir.dt
OUT_C, IN_C = 256, 256
KH = KW = 3
KK = KH * KW            # 9
P = 128
CC = IN_C // P          # 2 in-channel chunks
OO = OUT_C // P         # 2 out-channel chunks
CKK = IN_C * KK         # 2304
QKO = KK * P            # 1152 transposed cols per (oo, cc) quarter
DR = mybir.MatmulPerfMode.DoubleRow

BATCH, H, W = 32, 56, 56
HW = H * W
N_CORES = 8
IMGS = BATCH // N_CORES  # 4
HCH = 8                  # output rows per PSUM chunk (steady state)
NCH = H // HCH           # 7

# image-0 x slabs (row ranges) and chunk row ranges
SLABS0 = [(0, 5), (5, 13), (13, 21), (21, 29), (29, 37), (37, 45), (45, 56)]
CHUNKS0 = [(0, 4), (4, 8)] + [(8 * k, 8 * k + 8) for k in range(1, NCH)]


def _build(imgs=IMGS, psum_bufs=8, ostage=4):
    nc = bacc.Bacc("TRN2", target_bir_lowering=False, debug=False,
                   enable_asserts=False, num_devices=8)
    x = nc.declare_dram_parameter("x", [imgs, IN_C, HW], dt.float32, isOutput=False)
    w = nc.declare_dram_parameter("w", [OUT_C, CKK], dt.bfloat16, isOutput=False)
    wt = nc.declare_dram_parameter("wt", [OO, CKK, P], dt.bfloat16, isOutput=False)
    y = nc.declare_dram_parameter("y", [imgs, OUT_C, HW], dt.bfloat16, isOutput=True)

    # Transposed-weight view: [oo][c (partition, stride 9 rows)][cc][(k o) contig]
    wtv = wt.rearrange("oo (cc c k) o -> oo c cc (k o)", cc=CC, c=P, k=KK)

    with tile.TileContext(nc) as tc, ExitStack() as ctx:
        wp = ctx.enter_context(tc.tile_pool(name="wp", bufs=1))
        w_sb = wp.tile([P, OO, CKK], dt.bfloat16)         # original layout (scale)
        wt32 = wp.tile([P, OO, CC, QKO], dt.bfloat16)     # transposed
        wt8 = wp.tile([P, OO, CC, QKO], dt.float8e4)      # [c, oo, cc, (k o)]
        scale = wp.tile([P, OO], dt.float32)
        warm = wp.tile([P, P], dt.bfloat16)
        wt8v = wt8.rearrange("p oo cc (k o) -> p oo cc k o", o=P)

        x32p = ctx.enter_context(tc.tile_pool(name="x32", bufs=2))
        xqp = ctx.enter_context(tc.tile_pool(name="xq", bufs=imgs))
        pp = ctx.enter_context(tc.tile_pool(name="ps", bufs=psum_bufs, space="PSUM"))
        op = ctx.enter_context(tc.tile_pool(name="st", bufs=ostage))
        rawp = ctx.enter_context(tc.tile_pool(name="raw", bufs=2 * len(CHUNKS0)))

        x32s, xqs = {}, {}

        def wdma(oo, cc):
            nc.sync.dma_start(out=wt32[:, oo, cc, :], in_=wtv[oo, :, cc, :])

        def wsgn(oo, cc):
            # sign as (w >= 0) - 0.5 in {-0.5, +0.5} (exact fp8)
            nc.vector.tensor_scalar(
                out=wt8[:, oo, cc, :], in0=wt32[:, oo, cc, :],
                scalar1=0.0, scalar2=0.5,
                op0=mybir.AluOpType.is_ge, op1=mybir.AluOpType.subtract)

        def wodma(oo, half):
            q = CKK // 2
            nc.sync.dma_start(out=w_sb[:, oo, half * q:(half + 1) * q],
                              in_=w[oo * P:(oo + 1) * P, half * q:(half + 1) * q])

        def reduce_scale(oo):
            nc.vector.tensor_reduce(
                out=scale[:, oo:oo + 1], in_=w_sb[:, oo, :],
                axis=mybir.AxisListType.X,
                op=mybir.AluOpType.add, apply_absolute_value=True)
            # x2 compensates the +-0.5 sign weights
            nc.vector.tensor_scalar_mul(scale[:, oo:oo + 1], scale[:, oo:oo + 1],
                                        2.0 / CKK)

        def xalloc(img):
            x32s[img] = x32p.tile([P, CC, HW], dt.float32, name=f"x32_{img}", tag="x32")
            xqs[img] = xqp.tile([P, 2, CC, H, W], dt.float8e4, name=f"xq_{img}", tag="xq")

        def xdma(img, r0, r1):
            nc.sync.dma_start(
                out=x32s[img][:, :, r0 * W:r1 * W],
                in_=x[img, :, r0 * W:r1 * W].rearrange("(cc p) hw -> p cc hw", cc=CC))

        def quant(img, r0, r1, sub_eng, hi_eng=None):
            xq, x32 = xqs[img], x32s[img]
            x4 = x32.rearrange("p cc (h w) -> p cc h w", w=W)
            if hi_eng is None:
                nc.scalar.copy(out=xq[:, 0, :, r0:r1, :], in_=x4[:, :, r0:r1, :])
            else:
                hi_eng.tensor_copy(out=xq[:, 0, :, r0:r1, :], in_=x4[:, :, r0:r1, :])
            sub_eng.tensor_sub(xq[:, 1, :, r0:r1, :], x4[:, :, r0:r1, :],
                               xq[:, 0, :, r0:r1, :])

        def matmuls(img, oo, r0, r1):
            xq = xqs[img]
            rr = r1 - r0
            ps = pp.tile([P, rr, W], dt.float32, name=f"ps_{img}_{oo}_{r0}", tag="ps")
            n, last = 0, CC * KK - 1
            for ki in range(KH):
                for cc in range(CC):
                    for kj in range(KW):
                        xr = r0 + ki - 1
                        ro = max(0, -xr)
                        rows = min(H, xr + rr) - (xr + ro)
                        xc = kj - 1
                        co = max(0, -xc)
                        cols = min(W, xc + W) - (xc + co)
                        nc.tensor.matmul(
                            ps[:, ro:ro + rows, co:co + cols],
                            lhsT=wt8v[:, oo, cc, ki * KW + kj, :]
                                .unsqueeze(1).broadcast_to([P, 2, P]),
                            rhs=xq[:, :, cc, xr + ro:xr + ro + rows,
                                   xc + co:xc + co + cols],
                            start=(n == 0), stop=(n == last),
                            perf_mode=DR, skip_group_check=True)
                        n += 1
            return ps.rearrange("p h w -> p (h w)")

        raw_sts = []

        def conv_chunk_raw(img, oo, r0, r1, eng):
            # evict unscaled (no scale dependency); fixed up in place later
            psf = matmuls(img, oo, r0, r1)
            st = rawp.tile([P, (r1 - r0) * W], dt.bfloat16,
                           name=f"raw_{img}_{oo}_{r0}", tag="raw")
            if eng is nc.scalar:
                eng.copy(out=st, in_=psf)
            else:
                eng.tensor_copy(out=st, in_=psf)
            raw_sts.append((img, oo, r0, r1, st))

        def fixup(img, oo, r0, r1, st, eng):
            if eng is nc.scalar:
                eng.mul(st, st, scale[:, oo:oo + 1])
            else:
                eng.tensor_scalar_mul(st, st, scale[:, oo:oo + 1])
            nc.sync.dma_start(
                out=y[img, oo * P:(oo + 1) * P, r0 * W:r1 * W], in_=st)

        evn = [0]

        def conv_chunk2(img, oo, r0, r1):
            psf = matmuls(img, oo, r0, r1)
            st = op.tile([P, (r1 - r0) * W], dt.bfloat16,
                         name=f"st_{img}_{oo}_{r0}r", tag="st")
            eng = (nc.vector, nc.scalar)[evn[0] % 2]
            evn[0] += 1
            if eng is nc.scalar:
                eng.mul(st, psf, scale[:, oo:oo + 1])
            else:
                eng.tensor_scalar_mul(st, psf, scale[:, oo:oo + 1])
            nc.sync.dma_start(
                out=y[img, oo * P:(oo + 1) * P, r0 * W:r1 * W], in_=st)

        def conv_chunk(img, oo, ih):
            psf = matmuls(img, oo, ih * HCH, ih * HCH + HCH)
            st = op.tile([P, HCH * W], dt.bfloat16, name=f"st_{img}_{oo}_{ih}", tag="st")
            eng = (nc.vector, nc.scalar)[evn[0] % 2]
            evn[0] += 1
            if eng is nc.scalar:
                eng.mul(st, psf, scale[:, oo:oo + 1])
            else:
                eng.tensor_scalar_mul(st, psf, scale[:, oo:oo + 1])
            nc.sync.dma_start(
                out=y[img, oo * P:(oo + 1) * P, ih * HCH * W:(ih + 1) * HCH * W],
                in_=st)

        # --- emission schedule ---
        # PE p-state warmup: dummy accumulation on a zeroed tile, result unread.
        nc.vector.memset(warm, 0.0)
        dps = pp.tile([P, P], dt.float32, name="ps_warm", tag="ps")
        NWARM = 24
        for i in range(NWARM):
            nc.tensor.matmul(dps, lhsT=warm, rhs=warm,
                             start=(i == 0), stop=(i == NWARM - 1),
                             skip_group_check=True)

        # image-0 load stream with conv pairs interleaved (Act/DVE queues stay
        # in ready order); weight tiles ride between the x slabs.
        xalloc(0)
        xdma(0, *SLABS0[0])
        wdma(0, 0)
        wdma(0, 1)
        wsgn(0, 0)
        quant(0, *SLABS0[0], nc.vector, hi_eng=nc.vector)
        wsgn(0, 1)
        xdma(0, *SLABS0[1])
        wdma(1, 0)
        quant(0, *SLABS0[1], nc.vector)
        wsgn(1, 0)
        for pair, (r0, r1) in enumerate(CHUNKS0):
            s = pair + 2
            if s < len(SLABS0):
                xdma(0, *SLABS0[s])
                if s == 2:
                    wdma(1, 1)
                quant(0, *SLABS0[s], nc.vector)
                if s == 2:
                    wsgn(1, 1)
            conv_chunk_raw(0, 0, r0, r1, nc.scalar)
            conv_chunk_raw(0, 1, r0, r1, nc.vector)

        # image 1 x + scale weights + reduction + image-0 fixups
        xalloc(1)
        for hf in range(2):
            xdma(1, 28 * hf, 28 * hf + 28)
            quant(1, 28 * hf, 28 * hf + 28, nc.vector)
        for oo in range(OO):
            for half in range(2):
                wodma(oo, half)
        reduce_scale(0)
        reduce_scale(1)
        for i, (img, oo, r0, r1, st) in enumerate(raw_sts):
            fixup(img, oo, r0, r1, st, (nc.scalar, nc.vector)[i % 2])

        def load_img(img):
            xalloc(img)
            for hf in range(2):
                xdma(img, 28 * hf, 28 * hf + 28)
                quant(img, 28 * hf, 28 * hf + 28, nc.gpsimd)

        load_img(2)
        for ih in range(NCH):
            conv_chunk(1, 0, ih)
        load_img(3)
        for ih in range(NCH):
            conv_chunk(1, 1, ih)
        for img in range(2, imgs):
            for oo in range(OO):
                for ih in range(NCH):
                    if img == imgs - 1 and oo == OO - 1 and ih == NCH - 1:
                        conv_chunk2(img, oo, 48, 52)
                        conv_chunk2(img, oo, 52, 56)
                    else:
                        conv_chunk(img, oo, ih)
    nc.compile()
    return nc


_NC_CACHE = {}


def _get_nc():
    if "nc" not in _NC_CACHE:
        _NC_CACHE["nc"] = _build()
    return _NC_CACHE["nc"]


def kernel(**inputs) -> np.ndarray:
    import ml_dtypes
    from concourse.bass_utils import run_bass_kernel_spmd

    x = np.ascontiguousarray(np.asarray(inputs["x"], dtype=np.float32))
    weight = np.ascontiguousarray(np.asarray(inputs["weight"], dtype=np.float32))
    assert x.shape == (BATCH, IN_C, H, W), x.shape
    assert weight.shape == (OUT_C * CKK, 1), weight.shape

    bf16 = ml_dtypes.bfloat16
    w2d = np.ascontiguousarray(weight.reshape(OUT_C, CKK).astype(bf16))
    # host-side layout transpose + bf16 narrowing only: [OO, CKK, P]
    wtr = np.ascontiguousarray(
        weight.reshape(OO, P, CKK).transpose(0, 2, 1).astype(bf16))

    nc = _get_nc()
    xr = x.reshape(BATCH, IN_C, HW)
    in_maps = [
        {"x": xr[c * IMGS:(c + 1) * IMGS], "w": w2d, "wt": wtr}
        for c in range(N_CORES)
    ]
    res = run_bass_kernel_spmd(nc, in_maps, core_ids=list(range(N_CORES)))
    out = np.concatenate(
        [np.asarray(res.results[c]["y"]).astype(np.float32) for c in range(N_CORES)],
        axis=0)
    return out.reshape(BATCH, OUT_C, H, W)
